# revision 16
# baseline (speedup 1.0000x reference)
"""Trainium2 Bass kernel for linear attention over external memory.

Computes out = x @ (keys^T @ vals) for
  x [4, 2048, 1024] f32, keys/vals [65536, 1024] f32.

Sharding across 8 NeuronCores: keys/vals sharded along the memory dim M
(8192 rows per core); each core computes a partial kv = keys_s^T @ vals_s,
AllReduces kv in fp16 (column-split, pipelined against the end of stage 1),
then computes its token shard of x @ kv (x sharded by token).

Stage 1 streams keys/vals as fp32r (TF32-like, full PE rate for moving
dim >= 256) directly from DMA.  To let the first AllReduce fire early,
the LAST W chunks only compute the first column split while streaming;
their data (keys full + vals' remaining columns) is cast to fp16 into a
small SBUF hold, and the remaining column splits run after the stream —
by then kv[:, split0] is complete and its AllReduce overlaps those
matmuls.  Stage 2 (x @ kv) runs in fp16 with x transposed via the DMA
XBAR (no PE transposes).
"""

import numpy as np

# Problem shapes (hardcoded per contract).
B, S, D = 4, 2048, 1024
M = 65536
NCORES = 8
P = 128
T = (B * S) // NCORES          # 1024 tokens per core
KM = M // NCORES               # 8192 memory rows per core
NCH = KM // P                  # 64 k-chunks
DB = D // P                    # 8 d-blocks
HALF = D // 2                  # 512
TCH = T // P                   # 8 token chunks
G = 8                          # chunks per front PSUM group
W = 16                         # deferred-tail chunks
FRONT = NCH - W                # 48
NG = FRONT // G                # 6 front groups

# Column splits for the kv AllReduce pipeline.  The stream phase covers
# SPLITS[0] for the last W chunks; SPLITS[1:] are deferred.
SPLITS = [(0, 512), (512, 1024)]

_CACHE = {}


def _build_nc():
    import concourse.bacc as bacc
    import concourse.tile as tile
    from concourse import mybir

    f32 = mybir.dt.float32
    f32r = mybir.dt.float32r
    f16 = mybir.dt.float16
    ACT_COPY = mybir.ActivationFunctionType.Copy
    ADD = mybir.AluOpType.add

    s_lo, s_hi = SPLITS[0]
    defer_w = D - s_hi            # deferred vals columns per chunk

    nc = bacc.Bacc("TRN2", target_bir_lowering=False, debug=False,
                   num_devices=NCORES)

    xs_d = nc.dram_tensor("xs", [T, D], f32, kind="ExternalInput")
    ks_d = nc.dram_tensor("ks", [KM, D], f32r, kind="ExternalInput")
    vs_d = nc.dram_tensor("vs", [KM, D], f32r, kind="ExternalInput")
    out_d = nc.dram_tensor("out", [T, D], f32, kind="ExternalOutput")

    ks_r = ks_d.ap().rearrange("(c p) n -> c p n", p=P)   # [64, 128, 1024]
    vs_r = vs_d.ap().rearrange("(c p) n -> c p n", p=P)
    xs_r = xs_d.ap().rearrange("(c p) n -> c p n", p=P)   # [8, 128, 1024]

    with tile.TileContext(nc) as tc:
        with (
            tc.tile_pool(name="const", bufs=1) as const,
            tc.tile_pool(name="kst", bufs=10) as kst,    # f32r staging
            tc.tile_pool(name="vst", bufs=10) as vst,
            tc.tile_pool(name="ktl", bufs=W) as ktl,     # fp16 tail hold
            tc.tile_pool(name="vtl", bufs=W) as vtl,
            tc.tile_pool(name="accp", bufs=DB) as accp,
            tc.tile_pool(name="xst", bufs=3) as xst,
            tc.tile_pool(name="xhp", bufs=3) as xhp,
            tc.tile_pool(name="xtp", bufs=DB) as xtp,
            tc.tile_pool(name="kvio", bufs=2) as kvio,
            tc.tile_pool(name="outp", bufs=2) as outp,
            tc.tile_pool(name="ps", bufs=8, space="PSUM") as ps,
            tc.tile_pool(name="dram", bufs=4 + 2 * len(SPLITS),
                         space="DRAM") as dram,
        ):
            # Warm-up collective: arms the ncfw collective stream so the
            # first real AllReduce doesn't pay the wake-up and RDH
            # cold-path costs (256KB is big enough to exercise RDH).
            warm = const.tile([P, 1024], f16)
            nc.gpsimd.memset(warm[:], 0.0)
            warm_in = dram.tile([P, 1024], f16, name="warm_in")
            warm_out = dram.tile([P, 1024], f16, name="warm_out",
                                 addr_space="Shared")
            nc.gpsimd.dma_start(out=warm_in[:], in_=warm[:])
            nc.gpsimd.collective_compute(
                "AllReduce",
                ADD,
                replica_groups=[list(range(NCORES))],
                ins=[warm_in.opt()],
                outs=[warm_out.opt()],
            )

            # kv accumulator (fp16): acc[j] = kv[j*128:(j+1)*128, :].
            acc = [accp.tile([P, D], f16, name=f"acc{j}", tag="acc")
                   for j in range(DB)]
            for j in range(DB):
                nc.vector.memset(acc[j][:], 0.0)

            kf_tiles = [None] * NCH
            vf_tiles = [None] * NCH
            kh_tiles = [None] * NCH   # fp16 tail hold (full columns)
            vh_tiles = [None] * NCH   # fp16 tail hold (deferred columns)

            def emit_chunk(c):
                """Stage chunk c (f32r).  Tail chunks additionally cast
                keys (Vector) and vals' deferred columns (Scalar) to the
                fp16 hold."""
                kf = kst.tile([P, D], f32r, name="kf", tag="kf")
                vf = vst.tile([P, D], f32r, name="vf", tag="vf")
                if c < 2:
                    # Split the first chunks' DMAs so the first matmul
                    # only waits on a 256KB transfer.
                    nc.sync.dma_start(out=kf[:, :HALF],
                                      in_=ks_r[c][:, :HALF])
                    nc.sync.dma_start(out=vf[:, :HALF],
                                      in_=vs_r[c][:, :HALF])
                    nc.sync.dma_start(out=kf[:, HALF:],
                                      in_=ks_r[c][:, HALF:])
                    nc.sync.dma_start(out=vf[:, HALF:],
                                      in_=vs_r[c][:, HALF:])
                else:
                    nc.sync.dma_start(out=kf[:], in_=ks_r[c])
                    nc.sync.dma_start(out=vf[:], in_=vs_r[c])
                kf_tiles[c] = kf
                vf_tiles[c] = vf
                if c >= FRONT:
                    # vals' deferred columns cast on Scalar; the trigger
                    # may stall the scalar queue until the staging DMA
                    # lands, but nothing behind it is time-critical.
                    vh = vtl.tile([P, defer_w], f16, name="vh", tag="vh")
                    nc.scalar.activation(vh[:],
                                         vf[:, s_hi:].bitcast(f32),
                                         ACT_COPY)
                    vh_tiles[c] = vh

            # x pipeline: load (sync q), cast to fp16 (Vector), transpose
            # via DMA XBAR (Scalar hwdge q).  xT[j] = x^T[d-block j, :].
            xT = [xtp.tile([P, T], f16, name=f"xT{j}", tag="xT")
                  for j in range(DB)]

            def emit_x(i):
                xf = xst.tile([P, D], f32, name="xf", tag="xf")
                nc.sync.dma_start(out=xf[:], in_=xs_r[i])
                xh = xhp.tile([P, D], f16, name="xh", tag="xh")
                nc.vector.tensor_copy(out=xh[:], in_=xf[:])
                for j in range(DB):
                    nc.scalar.dma_start(
                        out=xT[j][:, i * P:(i + 1) * P],
                        in_=xh[:, j * P:(j + 1) * P],
                        transpose=True)

            # ---- stage 1 front: 48 chunks, both halves per group ----
            next_chunk = 0
            while next_chunk < 2 * G:
                emit_chunk(next_chunk)
                next_chunk += 1

            for gi in range(NG):
                tgt = min((gi + 3) * G, NCH)
                while next_chunk < tgt:
                    emit_chunk(next_chunk)
                    next_chunk += 1
                # One x chunk per group: its DMA/cast/transposes never
                # wait on slow pool slots, so no queue head-blocking.
                if gi < TCH:
                    emit_x(gi)
                for h in range(2):
                    e0 = h * HALF
                    pst = [ps.tile([P, HALF], f32, name=f"kv{h}_{j}",
                                   tag="ps") for j in range(DB)]
                    for ci in range(G):
                        c = gi * G + ci
                        for j in range(DB):
                            nc.tensor.matmul(
                                pst[j][:],
                                kf_tiles[c][:, j * P:(j + 1) * P],
                                vf_tiles[c][:, e0:e0 + HALF],
                                start=(ci == 0), stop=(ci == G - 1))
                    for j in range(DB):
                        nc.vector.tensor_tensor(
                            out=acc[j][:, e0:e0 + HALF],
                            in0=pst[j][:],
                            in1=acc[j][:, e0:e0 + HALF],
                            op=ADD)
            for i in range(NG, TCH):
                emit_x(i)

            # Keys casts for the tail hold, emitted only now so they sit
            # BEHIND all front drains in the Vector queue (their staging
            # DMAs land mid-stream; emitting them earlier head-blocks
            # the drains and stalls the PSUM pipeline).
            for c in range(FRONT, NCH):
                kh = ktl.tile([P, D], f16, name="kh", tag="kh")
                nc.vector.tensor_copy(out=kh[:],
                                      in_=kf_tiles[c][:].bitcast(f32))
                kh_tiles[c] = kh

            # ---- stage 1 tail stream pass: SPLITS[0] from f32r ----
            sw = s_hi - s_lo
            pst = [ps.tile([P, sw], f32, name=f"kvs_{j}", tag="ps")
                   for j in range(DB)]
            for ci in range(W):
                c = FRONT + ci
                for j in range(DB):
                    nc.tensor.matmul(
                        pst[j][:],
                        kf_tiles[c][:, j * P:(j + 1) * P],
                        vf_tiles[c][:, s_lo:s_hi],
                        start=(ci == 0), stop=(ci == W - 1))
            for j in range(DB):
                nc.vector.tensor_tensor(
                    out=acc[j][:, s_lo:s_hi],
                    in0=pst[j][:],
                    in1=acc[j][:, s_lo:s_hi],
                    op=ADD)

            # AllReduce split 0 + deferred passes, each followed by its
            # AllReduce.  Readbacks interleave so each lands as soon as
            # its collective completes.
            bounce_out = [None] * len(SPLITS)
            kvr = [None] * len(SPLITS)

            def emit_ar(t):
                lo, hi = SPLITS[t]
                wd = hi - lo
                b_in = dram.tile([P, DB * wd], f16,
                                 name=f"bin{t}", tag="bin")
                b_out = dram.tile([P, DB * wd], f16,
                                  name=f"bout{t}", tag="bout",
                                  addr_space="Shared")
                for j in range(DB):
                    nc.gpsimd.dma_start(
                        out=b_in[:, j * wd:(j + 1) * wd],
                        in_=acc[j][:, lo:hi])
                nc.gpsimd.collective_compute(
                    "AllReduce",
                    ADD,
                    replica_groups=[list(range(NCORES))],
                    ins=[b_in.opt()],
                    outs=[b_out.opt()],
                )
                bounce_out[t] = b_out
                kvh = kvio.tile([P, DB * wd], f16, name=f"kvr{t}",
                                tag="kvio")
                for j in range(DB):
                    nc.gpsimd.dma_start(
                        out=kvh[:, j * wd:(j + 1) * wd],
                        in_=b_out[:, j * wd:(j + 1) * wd])
                kvr[t] = kvh

            emit_ar(0)

            # ---- deferred tail passes (fp16 hold) ----
            for t in range(1, len(SPLITS)):
                lo, hi = SPLITS[t]
                wd = hi - lo
                pst = [ps.tile([P, wd], f32, name=f"kvd{t}_{j}",
                               tag="ps") for j in range(DB)]
                for ci in range(W):
                    c = FRONT + ci
                    for j in range(DB):
                        nc.tensor.matmul(
                            pst[j][:],
                            kh_tiles[c][:, j * P:(j + 1) * P],
                            vh_tiles[c][:, lo - s_hi:hi - s_hi],
                            start=(ci == 0), stop=(ci == W - 1))
                for j in range(DB):
                    nc.vector.tensor_tensor(
                        out=acc[j][:, lo:hi],
                        in0=pst[j][:],
                        in1=acc[j][:, lo:hi],
                        op=ADD)
                emit_ar(t)

            # ---- stage 2: out = x @ kv, per split ----
            for t in range(len(SPLITS)):
                lo, hi = SPLITS[t]
                wd = hi - lo
                kvh = kvr[t]
                for i in range(TCH):
                    po = ps.tile([P, wd], f32, name="po", tag="ps")
                    for j in range(DB):
                        nc.tensor.matmul(
                            po[:],
                            xT[j][:, i * P:(i + 1) * P],
                            kvh[:, j * wd:(j + 1) * wd],
                            start=(j == 0), stop=(j == DB - 1))
                    ob = outp.tile([P, wd], f32, name="ob", tag="ob")
                    nc.scalar.activation(ob[:], po[:], ACT_COPY)
                    nc.scalar.dma_start(
                        out=out_d.ap()[i * P:(i + 1) * P, lo:hi],
                        in_=ob[:])

    nc.compile()
    return nc


def _get_nc():
    if "nc" not in _CACHE:
        _CACHE["nc"] = _build_nc()
    return _CACHE["nc"]


def kernel(**inputs):
    from concourse.bass_utils import run_bass_kernel_spmd

    x = np.ascontiguousarray(np.asarray(inputs["x"], dtype=np.float32))
    keys = np.ascontiguousarray(np.asarray(inputs["keys"], dtype=np.float32))
    vals = np.ascontiguousarray(np.asarray(inputs["vals"], dtype=np.float32))
    xf = x.reshape(B * S, D)

    nc = _get_nc()
    in_maps = []
    for c in range(NCORES):
        in_maps.append({
            "xs": xf[c * T:(c + 1) * T],
            "ks": keys[c * KM:(c + 1) * KM],
            "vs": vals[c * KM:(c + 1) * KM],
        })
    res = run_bass_kernel_spmd(nc, in_maps, list(range(NCORES)))
    out = np.concatenate([res.results[c]["out"] for c in range(NCORES)],
                         axis=0)
    return out.reshape(B, S, D).astype(np.float32)


# revision 17
# speedup vs baseline: 1.3778x; 1.3778x over previous
"""Trainium2 Bass kernel for linear attention over external memory.

Computes out = x @ (keys^T @ vals) for
  x [4, 2048, 1024] f32, keys/vals [65536, 1024] f32.

Sharding across 8 NeuronCores: keys/vals sharded along the memory dim M
(8192 rows per core); each core computes a partial kv = keys_s^T @ vals_s,
AllReduces kv in fp16 (split in two column halves), then computes its
token shard of x @ kv (x sharded by token, 1024 rows per core).

Stage 1 runs in float32r (TF32-like, full PE rate for moving dim >= 256)
directly on the DMA'd f32 data.  kv is accumulated in PSUM per group of
8 k-chunks and drained into an fp16 SBUF accumulator.

Tail restructure: the last W=16 chunks are processed one column half at
a time.  While they stream, their keys (full) and vals' second half are
cast to an fp16 SBUF hold; the h=0 pass runs from f32r staging, so the
h=0 AllReduce fires ~34us before stage-1 ends and overlaps the h=1 pass
(which runs from the fp16 hold).  The h=1 AllReduce then overlaps the x
transposes and stage 2 on the h=0 columns.
"""

import numpy as np

# Problem shapes (hardcoded per contract).
B, S, D = 4, 2048, 1024
M = 65536
NCORES = 8
P = 128
T = (B * S) // NCORES          # 1024 tokens per core
KM = M // NCORES               # 8192 memory rows per core
NC_ = KM // P                  # 64 k-chunks
G = 8                          # chunks per PSUM accumulation group
DB = D // P                    # 8 d-blocks
HALF = D // 2                  # 512
TCH = T // P                   # 8 token chunks
W = 16                         # tail chunks (h-split, fp16 hold)
FRONT = NC_ - W                # 48
NG = FRONT // G                # 6 front groups

_CACHE = {}


def _build_nc():
    import concourse.bacc as bacc
    import concourse.tile as tile
    from concourse import mybir
    from concourse.masks import make_identity

    f32 = mybir.dt.float32
    f32r = mybir.dt.float32r
    f16 = mybir.dt.float16
    ACT_COPY = mybir.ActivationFunctionType.Copy
    ADD = mybir.AluOpType.add

    nc = bacc.Bacc("TRN2", target_bir_lowering=False, debug=False,
                   num_devices=NCORES)

    xs_d = nc.dram_tensor("xs", [T, D], f32, kind="ExternalInput")
    ks_d = nc.dram_tensor("ks", [KM, D], f32r, kind="ExternalInput")
    vs_d = nc.dram_tensor("vs", [KM, D], f32r, kind="ExternalInput")
    out_d = nc.dram_tensor("out", [T, D], f32, kind="ExternalOutput")

    ks_r = ks_d.ap().rearrange("(c p) n -> c p n", p=P)   # [64, 128, 1024]
    vs_r = vs_d.ap().rearrange("(c p) n -> c p n", p=P)
    xs_r = xs_d.ap().rearrange("(c p) n -> c p n", p=P)   # [8, 128, 1024]

    with tile.TileContext(nc) as tc:
        with (
            tc.tile_pool(name="const", bufs=1) as const,
            tc.tile_pool(name="kfp", bufs=9) as kfp,
            tc.tile_pool(name="vfp", bufs=9) as vfp,
            tc.tile_pool(name="ktl", bufs=W) as ktl,
            tc.tile_pool(name="vtl", bufs=W) as vtl,
            tc.tile_pool(name="accp", bufs=2 * DB) as accp,
            tc.tile_pool(name="xstage", bufs=TCH) as xstage,
            tc.tile_pool(name="xtp", bufs=DB) as xtp,
            tc.tile_pool(name="kvio", bufs=2) as kvio,
            tc.tile_pool(name="outp", bufs=2) as outp,
            tc.tile_pool(name="ps", bufs=8, space="PSUM") as ps,
            tc.tile_pool(name="dram", bufs=8, space="DRAM") as dram,
        ):
            ident = const.tile([P, P], f32)
            make_identity(nc, ident)

            # Warm-up collective: arms the ncfw collective stream so the
            # first real AllReduce trigger doesn't pay the ~11us wake-up.
            warm = const.tile([P, 16], f16)
            nc.gpsimd.memset(warm[:], 0.0)
            warm_in = dram.tile([P, 16], f16, name="warm_in")
            warm_out = dram.tile([P, 16], f16, name="warm_out",
                                 addr_space="Shared")
            nc.gpsimd.dma_start(out=warm_in[:], in_=warm[:])
            nc.gpsimd.collective_compute(
                "AllReduce",
                ADD,
                replica_groups=[list(range(NCORES))],
                ins=[warm_in.opt()],
                outs=[warm_out.opt()],
            )

            # kv accumulator: tile (h*DB+j) holds kv[j*128:(j+1)*128,
            # h*512:(h+1)*512] as [128, 512] fp16.
            acc = [accp.tile([P, HALF], f16, name=f"acc{i}", tag="acc")
                   for i in range(2 * DB)]
            for i in range(2 * DB):
                nc.vector.memset(acc[i][:], 0.0)

            # ---- stage 1 front: 48 chunks in groups of 8, both halves --
            # keys/vals stream on the sync queue; first chunks row-split
            # across two DMAs to shorten the start ramp.
            kf_tiles = [None] * NC_
            vf_tiles = [None] * NC_

            def load_chunk(c):
                kt = kfp.tile([P, D], f32r, name="kt", tag="kt")
                vt = vfp.tile([P, D], f32r, name="vt", tag="vt")
                if c < 2:
                    nc.sync.dma_start(out=kt[:, :HALF],
                                      in_=ks_r[c][:, :HALF])
                    nc.sync.dma_start(out=vt[:, :HALF],
                                      in_=vs_r[c][:, :HALF])
                    nc.sync.dma_start(out=kt[:, HALF:],
                                      in_=ks_r[c][:, HALF:])
                    nc.sync.dma_start(out=vt[:, HALF:],
                                      in_=vs_r[c][:, HALF:])
                else:
                    nc.sync.dma_start(out=kt[:], in_=ks_r[c])
                    nc.sync.dma_start(out=vt[:], in_=vs_r[c])
                kf_tiles[c] = kt
                vf_tiles[c] = vt

            for g in range(NG):
                for ci in range(G):
                    load_chunk(g * G + ci)
                for h in range(2):
                    e0 = h * HALF
                    pst = [ps.tile([P, HALF], f32, name=f"kv{h}_{j}",
                                   tag="ps") for j in range(DB)]
                    for ci in range(G):
                        c = g * G + ci
                        for j in range(DB):
                            nc.tensor.matmul(
                                pst[j][:],
                                kf_tiles[c][:, j * P:(j + 1) * P],
                                vf_tiles[c][:, e0:e0 + HALF],
                                start=(ci == 0), stop=(ci == G - 1))
                    for j in range(DB):
                        nc.vector.tensor_tensor(
                            out=acc[h * DB + j][:],
                            in0=pst[j][:],
                            in1=acc[h * DB + j][:],
                            op=ADD)

            # Tail chunk loads continue the same stream.
            for c in range(FRONT, NC_):
                load_chunk(c)
            # x loads at the tail of the load stream.
            xf_tiles = []
            for i in range(TCH):
                xf = xstage.tile([P, D], f32, name="xf", tag="xf")
                nc.sync.dma_start(out=xf[:], in_=xs_r[i])
                xf_tiles.append(xf)

            # fp16 casts for the tail hold.  Emitted only now so they sit
            # behind all front drains in the Vector queue (their staging
            # DMAs land mid-stream; emitting them earlier head-blocks the
            # drains and stalls the PSUM pipeline).  keys (full) on
            # Vector, vals' second half on Scalar.
            kh_tiles = [None] * NC_
            vh_tiles = [None] * NC_
            for c in range(FRONT, NC_):
                kh = ktl.tile([P, D], f16, name="kh", tag="kh")
                nc.vector.tensor_copy(out=kh[:],
                                      in_=kf_tiles[c][:].bitcast(f32))
                kh_tiles[c] = kh
            for c in range(FRONT, NC_):
                vh = vtl.tile([P, HALF], f16, name="vh", tag="vh")
                nc.scalar.activation(vh[:],
                                     vf_tiles[c][:, HALF:].bitcast(f32),
                                     ACT_COPY)
                vh_tiles[c] = vh

            # ---- stage 1 tail, h=0 from f32r staging ----
            pst = [ps.tile([P, HALF], f32, name=f"kvt0_{j}", tag="ps")
                   for j in range(DB)]
            for ci in range(W):
                c = FRONT + ci
                for j in range(DB):
                    nc.tensor.matmul(
                        pst[j][:],
                        kf_tiles[c][:, j * P:(j + 1) * P],
                        vf_tiles[c][:, 0:HALF],
                        start=(ci == 0), stop=(ci == W - 1))
            for j in range(DB):
                nc.vector.tensor_tensor(
                    out=acc[j][:], in0=pst[j][:], in1=acc[j][:], op=ADD)

            # AllReduce h=0: DMA acc tiles straight into the bounce
            # buffer (fp16, no cast step).
            bounce_out = []
            for h in range(2):
                b_in = dram.tile([P, DB * HALF], f16,
                                 name=f"bin{h}", tag="bin")
                b_out = dram.tile([P, DB * HALF], f16,
                                  name=f"bout{h}", tag="bout",
                                  addr_space="Shared")
                bounce_out.append((b_in, b_out))

            def emit_ar(h):
                b_in, b_out = bounce_out[h]
                for j in range(DB):
                    nc.gpsimd.dma_start(
                        out=b_in[:, j * HALF:(j + 1) * HALF],
                        in_=acc[h * DB + j][:])
                nc.gpsimd.collective_compute(
                    "AllReduce",
                    ADD,
                    replica_groups=[list(range(NCORES))],
                    ins=[b_in.opt()],
                    outs=[b_out.opt()],
                )

            emit_ar(0)

            # ---- stage 1 tail, h=1 from the fp16 hold ----
            pst = [ps.tile([P, HALF], f32, name=f"kvt1_{j}", tag="ps")
                   for j in range(DB)]
            for ci in range(W):
                c = FRONT + ci
                for j in range(DB):
                    nc.tensor.matmul(
                        pst[j][:],
                        kh_tiles[c][:, j * P:(j + 1) * P],
                        vh_tiles[c][:],
                        start=(ci == 0), stop=(ci == W - 1))
            for j in range(DB):
                nc.vector.tensor_tensor(
                    out=acc[DB + j][:], in0=pst[j][:], in1=acc[DB + j][:],
                    op=ADD)
            emit_ar(1)

            # ---- x: PE-transpose, cast to fp16 (fills AR wait) ----
            xT = [xtp.tile([P, T], f16, name=f"xT{j}", tag="xT")
                  for j in range(DB)]
            for i in range(TCH):
                xf = xf_tiles[i]
                for j in range(DB):
                    pst = ps.tile([P, P], f32, name="pst", tag="ps")
                    nc.tensor.transpose(
                        pst[:], xf[:, j * P:(j + 1) * P], ident[:])
                    nc.vector.tensor_copy(
                        out=xT[j][:, i * P:(i + 1) * P], in_=pst[:])

            # ---- stage 2: out = x @ kv, per column half ----
            for h in range(2):
                kvh = kvio.tile([P, DB * HALF], f16, name=f"kvr{h}",
                                tag="kvio")
                for j in range(DB):
                    sl = slice(j * HALF, (j + 1) * HALF)
                    nc.gpsimd.dma_start(out=kvh[:, sl],
                                        in_=bounce_out[h][1][:, sl])
                for i in range(TCH):
                    po = ps.tile([P, HALF], f32, name="po", tag="ps")
                    for j in range(DB):
                        nc.tensor.matmul(
                            po[:],
                            xT[j][:, i * P:(i + 1) * P],
                            kvh[:, j * HALF:(j + 1) * HALF],
                            start=(j == 0), stop=(j == DB - 1))
                    ob = outp.tile([P, HALF], f32, name="ob", tag="ob")
                    nc.scalar.activation(ob[:], po[:], ACT_COPY)
                    nc.scalar.dma_start(
                        out=out_d.ap()[i * P:(i + 1) * P,
                                       h * HALF:(h + 1) * HALF],
                        in_=ob[:])

    nc.compile()
    return nc


def _get_nc():
    if "nc" not in _CACHE:
        _CACHE["nc"] = _build_nc()
    return _CACHE["nc"]


def kernel(**inputs):
    from concourse.bass_utils import run_bass_kernel_spmd

    x = np.ascontiguousarray(np.asarray(inputs["x"], dtype=np.float32))
    keys = np.ascontiguousarray(np.asarray(inputs["keys"], dtype=np.float32))
    vals = np.ascontiguousarray(np.asarray(inputs["vals"], dtype=np.float32))
    xf = x.reshape(B * S, D)

    nc = _get_nc()
    in_maps = []
    for c in range(NCORES):
        in_maps.append({
            "xs": xf[c * T:(c + 1) * T],
            "ks": keys[c * KM:(c + 1) * KM],
            "vs": vals[c * KM:(c + 1) * KM],
        })
    res = run_bass_kernel_spmd(nc, in_maps, list(range(NCORES)))
    out = np.concatenate([res.results[c]["out"] for c in range(NCORES)],
                         axis=0)
    return out.reshape(B, S, D).astype(np.float32)


# revision 22
# speedup vs baseline: 1.4224x; 1.0324x over previous
"""Trainium2 Bass kernel for linear attention over external memory.

Computes out = x @ (keys^T @ vals) for
  x [4, 2048, 1024] f32, keys/vals [65536, 1024] f32.

Sharding across 8 NeuronCores: keys/vals sharded along the memory dim M
(8192 rows per core); each core computes a partial kv = keys_s^T @ vals_s,
AllReduces kv in fp16 (split in two column halves), then computes its
token shard of x @ kv (x sharded by token, 1024 rows per core).

Stage 1 runs in float32r (TF32-like, full PE rate for moving dim >= 256)
directly on the DMA'd f32 data.  kv is accumulated in PSUM per group of
8 k-chunks and drained into an fp16 SBUF accumulator.

Tail restructure: the last W=16 chunks are processed one column half at
a time.  While they stream, their keys (full) and vals' second half are
cast to an fp16 SBUF hold; the h=0 pass runs from f32r staging, so the
h=0 AllReduce fires ~34us before stage-1 ends and overlaps the h=1 pass
(which runs from the fp16 hold).  The h=1 AllReduce then overlaps the x
transposes and stage 2 on the h=0 columns.
"""

import numpy as np

# Problem shapes (hardcoded per contract).
B, S, D = 4, 2048, 1024
M = 65536
NCORES = 8
P = 128
T = (B * S) // NCORES          # 1024 tokens per core
KM = M // NCORES               # 8192 memory rows per core
NC_ = KM // P                  # 64 k-chunks
G = 8                          # chunks per PSUM accumulation group
DB = D // P                    # 8 d-blocks
HALF = D // 2                  # 512
TCH = T // P                   # 8 token chunks
W = 12                         # tail chunks (h-split, fp16 hold)
FRONT = NC_ - W                # 52
FRONT_GROUPS = [4, 8, 8, 8, 8, 8, 8]
assert sum(FRONT_GROUPS) == FRONT

_CACHE = {}


def _build_nc():
    import concourse.bacc as bacc
    import concourse.tile as tile
    from concourse import mybir
    from concourse.masks import make_identity

    f32 = mybir.dt.float32
    f32r = mybir.dt.float32r
    f16 = mybir.dt.float16
    ACT_COPY = mybir.ActivationFunctionType.Copy
    ADD = mybir.AluOpType.add

    nc = bacc.Bacc("TRN2", target_bir_lowering=False, debug=False,
                   num_devices=NCORES)

    xs_d = nc.dram_tensor("xs", [T, D], f32, kind="ExternalInput")
    ks_d = nc.dram_tensor("ks", [KM, D], f32r, kind="ExternalInput")
    vs_d = nc.dram_tensor("vs", [KM, D], f32r, kind="ExternalInput")
    out_d = nc.dram_tensor("out", [T, D], f32, kind="ExternalOutput")

    ks_r = ks_d.ap().rearrange("(c p) n -> c p n", p=P)   # [64, 128, 1024]
    vs_r = vs_d.ap().rearrange("(c p) n -> c p n", p=P)
    xs_r = xs_d.ap().rearrange("(c p) n -> c p n", p=P)   # [8, 128, 1024]

    with tile.TileContext(nc) as tc:
        with (
            tc.tile_pool(name="const", bufs=1) as const,
            tc.tile_pool(name="kfp", bufs=10) as kfp,
            tc.tile_pool(name="vfp", bufs=11) as vfp,
            tc.tile_pool(name="ktl", bufs=W) as ktl,
            tc.tile_pool(name="vtl", bufs=W) as vtl,
            tc.tile_pool(name="accp", bufs=2 * DB) as accp,
            tc.tile_pool(name="xstage", bufs=TCH) as xstage,
            tc.tile_pool(name="xtp", bufs=DB) as xtp,
            tc.tile_pool(name="kvio", bufs=2) as kvio,
            tc.tile_pool(name="outp", bufs=2) as outp,
            tc.tile_pool(name="ps", bufs=8, space="PSUM") as ps,
            tc.tile_pool(name="dram", bufs=8, space="DRAM") as dram,
        ):
            ident = const.tile([P, P], f32)
            make_identity(nc, ident)

            # Warm-up collective: arms the ncfw collective stream so the
            # first real AllReduce trigger doesn't pay the ~11us wake-up.
            warm = const.tile([P, 16], f16)
            nc.gpsimd.memset(warm[:], 0.0)
            warm_in = dram.tile([P, 16], f16, name="warm_in")
            warm_out = dram.tile([P, 16], f16, name="warm_out",
                                 addr_space="Shared")
            nc.gpsimd.dma_start(out=warm_in[:], in_=warm[:])
            nc.gpsimd.collective_compute(
                "AllReduce",
                ADD,
                replica_groups=[list(range(NCORES))],
                ins=[warm_in.opt()],
                outs=[warm_out.opt()],
            )

            # kv accumulator: tile (h*DB+j) holds kv[j*128:(j+1)*128,
            # h*512:(h+1)*512] as [128, 512] fp16.
            acc = [accp.tile([P, HALF], f16, name=f"acc{i}", tag="acc")
                   for i in range(2 * DB)]
            for i in range(2 * DB):
                nc.vector.memset(acc[i][:], 0.0)

            # ---- stage 1 front: 48 chunks in groups of 8, both halves --
            # keys/vals stream on the sync queue; first chunks row-split
            # across two DMAs to shorten the start ramp.
            kf_tiles = [None] * NC_
            vf_tiles = [None] * NC_

            def load_chunk(c):
                kt = kfp.tile([P, D], f32r, name="kt", tag="kt")
                vt = vfp.tile([P, D], f32r, name="vt", tag="vt")
                if c < 2:
                    nc.sync.dma_start(out=kt[:, :HALF],
                                      in_=ks_r[c][:, :HALF])
                    nc.sync.dma_start(out=vt[:, :HALF],
                                      in_=vs_r[c][:, :HALF])
                    nc.sync.dma_start(out=kt[:, HALF:],
                                      in_=ks_r[c][:, HALF:])
                    nc.sync.dma_start(out=vt[:, HALF:],
                                      in_=vs_r[c][:, HALF:])
                else:
                    nc.sync.dma_start(out=kt[:], in_=ks_r[c])
                    nc.sync.dma_start(out=vt[:], in_=vs_r[c])
                kf_tiles[c] = kt
                vf_tiles[c] = vt

            c0 = 0
            for gsz in FRONT_GROUPS:
                for ci in range(gsz):
                    load_chunk(c0 + ci)
                for h in range(2):
                    e0 = h * HALF
                    pst = [ps.tile([P, HALF], f32, name=f"kv{h}_{j}",
                                   tag="ps") for j in range(DB)]
                    for ci in range(gsz):
                        c = c0 + ci
                        for j in range(DB):
                            nc.tensor.matmul(
                                pst[j][:],
                                kf_tiles[c][:, j * P:(j + 1) * P],
                                vf_tiles[c][:, e0:e0 + HALF],
                                start=(ci == 0), stop=(ci == gsz - 1))
                    for j in range(DB):
                        nc.vector.tensor_tensor(
                            out=acc[h * DB + j][:],
                            in0=pst[j][:],
                            in1=acc[h * DB + j][:],
                            op=ADD)
                c0 += gsz

            # Tail chunk loads continue the same stream.
            for c in range(FRONT, NC_):
                load_chunk(c)
            # x loads at the tail of the load stream.
            xf_tiles = []
            for i in range(TCH):
                xf = xstage.tile([P, D], f32, name="xf", tag="xf")
                nc.sync.dma_start(out=xf[:], in_=xs_r[i])
                xf_tiles.append(xf)

            # fp16 casts for the tail hold.  Emitted only now so they sit
            # behind all front drains in the Vector queue (their staging
            # DMAs land mid-stream; emitting them earlier head-blocks the
            # drains and stalls the PSUM pipeline).  keys (full) on
            # Vector, vals' second half on Scalar.
            kh_tiles = [None] * NC_
            vh_tiles = [None] * NC_
            for c in range(FRONT, NC_):
                kh = ktl.tile([P, D], f16, name="kh", tag="kh")
                nc.vector.tensor_copy(out=kh[:],
                                      in_=kf_tiles[c][:].bitcast(f32))
                kh_tiles[c] = kh
            for c in range(FRONT, NC_):
                vh = vtl.tile([P, HALF], f16, name="vh", tag="vh")
                nc.scalar.activation(vh[:],
                                     vf_tiles[c][:, HALF:].bitcast(f32),
                                     ACT_COPY)
                vh_tiles[c] = vh

            # ---- stage 1 tail, h=0 from f32r staging ----
            pst = [ps.tile([P, HALF], f32, name=f"kvt0_{j}", tag="ps")
                   for j in range(DB)]
            for ci in range(W):
                c = FRONT + ci
                for j in range(DB):
                    nc.tensor.matmul(
                        pst[j][:],
                        kf_tiles[c][:, j * P:(j + 1) * P],
                        vf_tiles[c][:, 0:HALF],
                        start=(ci == 0), stop=(ci == W - 1))
            for j in range(DB):
                nc.vector.tensor_tensor(
                    out=acc[j][:], in0=pst[j][:], in1=acc[j][:], op=ADD)

            # AllReduce h=0: DMA acc tiles straight into the bounce
            # buffer (fp16, no cast step).
            bounce_out = []
            for h in range(2):
                b_in = dram.tile([P, DB * HALF], f16,
                                 name=f"bin{h}", tag="bin")
                b_out = dram.tile([P, DB * HALF], f16,
                                  name=f"bout{h}", tag="bout",
                                  addr_space="Shared")
                bounce_out.append((b_in, b_out))

            def emit_ar(h):
                b_in, b_out = bounce_out[h]
                for j in range(DB):
                    nc.gpsimd.dma_start(
                        out=b_in[:, j * HALF:(j + 1) * HALF],
                        in_=acc[h * DB + j][:])
                nc.gpsimd.collective_compute(
                    "AllReduce",
                    ADD,
                    replica_groups=[list(range(NCORES))],
                    ins=[b_in.opt()],
                    outs=[b_out.opt()],
                )

            emit_ar(0)

            # ---- stage 1 tail, h=1 from the fp16 hold ----
            pst = [ps.tile([P, HALF], f32, name=f"kvt1_{j}", tag="ps")
                   for j in range(DB)]
            for ci in range(W):
                c = FRONT + ci
                for j in range(DB):
                    nc.tensor.matmul(
                        pst[j][:],
                        kh_tiles[c][:, j * P:(j + 1) * P],
                        vh_tiles[c][:],
                        start=(ci == 0), stop=(ci == W - 1))
            for j in range(DB):
                nc.vector.tensor_tensor(
                    out=acc[DB + j][:], in0=pst[j][:], in1=acc[DB + j][:],
                    op=ADD)
            emit_ar(1)

            # ---- x: PE-transpose, cast to fp16 (fills AR wait) ----
            xT = [xtp.tile([P, T], f16, name=f"xT{j}", tag="xT")
                  for j in range(DB)]
            for i in range(TCH):
                xf = xf_tiles[i]
                for j in range(DB):
                    pst = ps.tile([P, P], f32, name="pst", tag="ps")
                    nc.tensor.transpose(
                        pst[:], xf[:, j * P:(j + 1) * P], ident[:])
                    nc.vector.tensor_copy(
                        out=xT[j][:, i * P:(i + 1) * P], in_=pst[:])

            # ---- stage 2: out = x @ kv, per column half ----
            for h in range(2):
                kvh = kvio.tile([P, DB * HALF], f16, name=f"kvr{h}",
                                tag="kvio")
                for j in range(DB):
                    sl = slice(j * HALF, (j + 1) * HALF)
                    nc.gpsimd.dma_start(out=kvh[:, sl],
                                        in_=bounce_out[h][1][:, sl])
                for i in range(TCH):
                    po = ps.tile([P, HALF], f32, name="po", tag="ps")
                    for j in range(DB):
                        nc.tensor.matmul(
                            po[:],
                            xT[j][:, i * P:(i + 1) * P],
                            kvh[:, j * HALF:(j + 1) * HALF],
                            start=(j == 0), stop=(j == DB - 1))
                    ob = outp.tile([P, HALF], f32, name="ob", tag="ob")
                    nc.scalar.activation(ob[:], po[:], ACT_COPY)
                    nc.scalar.dma_start(
                        out=out_d.ap()[i * P:(i + 1) * P,
                                       h * HALF:(h + 1) * HALF],
                        in_=ob[:])

    nc.compile()
    return nc


def _get_nc():
    if "nc" not in _CACHE:
        _CACHE["nc"] = _build_nc()
    return _CACHE["nc"]


def kernel(**inputs):
    from concourse.bass_utils import run_bass_kernel_spmd

    x = np.ascontiguousarray(np.asarray(inputs["x"], dtype=np.float32))
    keys = np.ascontiguousarray(np.asarray(inputs["keys"], dtype=np.float32))
    vals = np.ascontiguousarray(np.asarray(inputs["vals"], dtype=np.float32))
    xf = x.reshape(B * S, D)

    nc = _get_nc()
    in_maps = []
    for c in range(NCORES):
        in_maps.append({
            "xs": xf[c * T:(c + 1) * T],
            "ks": keys[c * KM:(c + 1) * KM],
            "vs": vals[c * KM:(c + 1) * KM],
        })
    res = run_bass_kernel_spmd(nc, in_maps, list(range(NCORES)))
    out = np.concatenate([res.results[c]["out"] for c in range(NCORES)],
                         axis=0)
    return out.reshape(B, S, D).astype(np.float32)
